# Initial kernel scaffold
#
"""Trainium2 Bass kernel for a pre-norm transformer block (attention + MLP).

Sharding: batch (2) x query-block (4) across 8 cores. Each core computes
LN1 + K/V over its full batch (replicated within its 4-core group) and
attention / projection / MLP for its own 1024 query tokens. No collectives.

Device layouts (per core):
  xT  : LN1(x) feature-major [128c, 4ct, 4096t] bf16
  kT  : per head-pair hp [128 (2 heads x 64d), 4096m] bf16
  v   : token-major [128m, 32mt, 8h, 65] bf16 (65th col = ones -> softmax denom)
  S^T : [128m, 2h, 512n] PSUM (keys on partitions; head pair row-packed)
  A@V : out^T[65, n]: lhsT=[V|1] per head, accumulated over 32 m-tiles
"""

import numpy as np
import ml_dtypes

B, N, C = 2, 4096, 512
H, D = 8, 64
HID = 2048
NQ = 1024
NCORES = 8
EPS = 1e-5
BF = ml_dtypes.bfloat16

_CACHE = {}


def _build_program(repeat=1):
    from concourse import bacc
    import concourse.bass as bass
    import concourse.mybir as mybir
    from concourse.tile import TileContext

    dt = mybir.dt
    AF = mybir.ActivationFunctionType
    ALU = mybir.AluOpType

    nc = bacc.Bacc(None, target_bir_lowering=False)

    xfull = nc.dram_tensor("xfull", (N, C), dt.float32, kind="ExternalInput")
    xq = nc.dram_tensor("xq", (NQ, C), dt.float32, kind="ExternalInput")
    wq_d = nc.dram_tensor("wq_d", (128, 4, C), dt.bfloat16, kind="ExternalInput")
    wk_d = nc.dram_tensor("wk_d", (128, 4, C), dt.bfloat16, kind="ExternalInput")
    wv_d = nc.dram_tensor("wv_d", (128, 4, C), dt.bfloat16, kind="ExternalInput")
    wp_d = nc.dram_tensor("wp_d", (64, 8, C), dt.bfloat16, kind="ExternalInput")
    w1_d = nc.dram_tensor("w1_d", (128, 4, HID), dt.bfloat16, kind="ExternalInput")
    w2_d = nc.dram_tensor("w2_d", (128, 16, C), dt.bfloat16, kind="ExternalInput")
    bq_d = nc.dram_tensor("bq_d", (128, 4), dt.float32, kind="ExternalInput")
    bk_d = nc.dram_tensor("bk_d", (128, 4), dt.float32, kind="ExternalInput")
    bv_d = nc.dram_tensor("bv_d", (C,), dt.float32, kind="ExternalInput")
    bp_d = nc.dram_tensor("bp_d", (C,), dt.float32, kind="ExternalInput")
    b1_d = nc.dram_tensor("b1_d", (128, 16), dt.float32, kind="ExternalInput")
    b2_d = nc.dram_tensor("b2_d", (C,), dt.float32, kind="ExternalInput")
    y = nc.dram_tensor("y", (NQ, C), dt.float32, kind="ExternalOutput")

    xfull_t = xfull.rearrange("(i p) c -> p i c", p=128)
    xq_t = xq.rearrange("(i p) c -> p i c", p=128)
    y_t = y.rearrange("(i p) c -> p i c", p=128)

    import contextlib
    with TileContext(nc) as tc:
      with (tc.For_i(0, repeat, 1) if repeat > 1 else contextlib.nullcontext()):
        R = "r0_"
        with tc.tile_pool(name=R + "pers", bufs=1) as pers, \
             tc.tile_pool(name=R + "stat", bufs=4) as statp, \
             tc.tile_pool(name=R + "stream", bufs=3) as stream, \
             tc.tile_pool(name=R + "pB", bufs=1) as pB, \
             tc.tile_pool(name=R + "kq", bufs=2) as kqp, \
             tc.tile_pool(name=R + "ptp", bufs=3) as ptp, \
             tc.tile_pool(name=R + "pall", bufs=1, space="PSUM") as pall:

            eps_t = pers.tile([128, 1], dt.float32, name=R + "eps")
            nc.vector.memset(eps_t, EPS)
            ones_sb = pers.tile([128, 64], dt.float32, name=R + "ones")
            nc.vector.memset(ones_sb, 1.0)
            xq_sb = pers.tile([128, 8, C], dt.float32, name=R + "xq_sb")
            nc.sync.dma_start(out=xq_sb, in_=xq_t[:])
            bq_sb = pers.tile([128, 4], dt.float32, name=R + "bq_sb")
            bk_sb = pers.tile([128, 4], dt.float32, name=R + "bk_sb")
            bv_sb = pers.tile([128, 8, 64], dt.bfloat16, name=R + "bv_sb")
            bp_sb = pers.tile([128, C], dt.float32, name=R + "bp_sb")
            b1_sb = pers.tile([128, 16], dt.float32, name=R + "b1_sb")
            b2_sb = pers.tile([128, C], dt.float32, name=R + "b2_sb")
            nc.sync.dma_start(out=bq_sb, in_=bq_d[:])
            nc.sync.dma_start(out=bk_sb, in_=bk_d[:])
            nc.sync.dma_start(out=b1_sb, in_=b1_d[:])
            nc.gpsimd.dma_start(out=bv_sb, in_=bass.AP(tensor=bv_d, offset=0, ap=[[0, 128], [1, C]]))
            nc.gpsimd.dma_start(out=bp_sb, in_=bass.AP(tensor=bp_d, offset=0, ap=[[0, 128], [1, C]]))
            nc.gpsimd.dma_start(out=b2_sb, in_=bass.AP(tensor=b2_d, offset=0, ap=[[0, 128], [1, C]]))

            # phase-B persistent tiles (attention)
            wp_sb = pB.tile([64, 8, C], dt.bfloat16, name=R + "wp_sb")
            nc.gpsimd.dma_start(out=wp_sb, in_=wp_d[:])
            v_sb = pB.tile([128, 32, H, 65], dt.bfloat16, name=R + "v_sb")
            nc.vector.memset(v_sb[:, :, :, 64:65], 1.0)
            ao_lo = pB.tile([64, 4, NQ], dt.bfloat16, name=R + "ao_lo")
            ao_hi = pB.tile([64, 4, NQ], dt.bfloat16, name=R + "ao_hi")

            def ln_group(src_dram_or_sb, g, xT, tag, from_sbuf=False):
                """4 token-tiles: 1 load, per-tile LN stats+normalize, 1 batched transpose.

                xT layout: [128 ci, nblk, 4 co, 128 t]."""
                eng = nc.sync if g % 2 == 0 else nc.scalar
                if from_sbuf:
                    xt4 = src_dram_or_sb
                else:
                    xt4 = stream.tile([128, 4, C], dt.float32, tag="lnx", bufs=2, name=f"{R}{tag}x{g}")
                    eng.dma_start(out=xt4, in_=src_dram_or_sb)
                xn4 = stream.tile([128, 4, C], dt.bfloat16, tag="lnn", bufs=3, name=f"{R}{tag}n{g}")
                for j in range(4):
                    i = 4 * g + j
                    stats = statp.tile([128, 6], dt.float32, tag="lnst", name=f"{R}{tag}st{i}")
                    mv = statp.tile([128, 2], dt.float32, tag="lnmv", name=f"{R}{tag}mv{i}")
                    nc.vector.bn_stats(stats, xt4[:, j, :])
                    nc.vector.bn_aggr(mv, stats)
                    sq = statp.tile([128, 1], dt.float32, tag="lnsq", name=f"{R}{tag}sq{i}")
                    nc.scalar.activation(sq, mv[:, 1:2], AF.Sqrt, bias=eps_t)
                    rstd = statp.tile([128, 1], dt.float32, tag="lnrs", name=f"{R}{tag}rs{i}")
                    nc.vector.reciprocal(rstd, sq)
                    nmr = statp.tile([128, 1], dt.float32, tag="lnnm", name=f"{R}{tag}nm{i}")
                    nc.vector.tensor_scalar(nmr, mv[:, 0:1], rstd, -1.0, ALU.mult, ALU.mult)
                    nc.scalar.activation(xn4[:, j, :], xt4[:, j, :], AF.Identity, bias=nmr, scale=rstd)
                eng.dma_start(out=xT[:, 4 * g:4 * g + 4, :, :], in_=xn4, transpose=True)

            def emit_kq(hp, kT, qT, wk_sb, wq_sb, xT, xqT):
                for nch in range(8):
                    ps_k = pall.tile([128, 512], dt.float32, bufs=2, tag="psmall",
                                     name=f"{R}ps_k{hp}_{nch}")
                    for kt in range(4):
                        nc.tensor.matmul(ps_k, wk_sb[:, kt, hp * 128:(hp + 1) * 128],
                                         xT[:, 4 * nch:4 * nch + 4, kt, :],
                                         start=(kt == 0), stop=(kt == 3))
                    nc.vector.tensor_scalar(kT[:, nch * 512:(nch + 1) * 512],
                                            ps_k, bk_sb[:, hp:hp + 1], None, ALU.add)
                for nch in range(2):
                    ps_q = pall.tile([128, 512], dt.float32, bufs=2, tag="psmall",
                                     name=f"{R}ps_q{hp}_{nch}")
                    for kt in range(4):
                        nc.tensor.matmul(ps_q, wq_sb[:, kt, hp * 128:(hp + 1) * 128],
                                         xqT[:, 4 * nch:4 * nch + 4, kt, :],
                                         start=(kt == 0), stop=(kt == 3))
                    nc.vector.tensor_scalar(qT[:, nch * 512:(nch + 1) * 512],
                                            ps_q, bq_sb[:, hp:hp + 1], None, ALU.add)

            def emit_attention(hp, kT, qT):
                for nch in range(2):
                    nsl = slice(nch * 512, (nch + 1) * 512)
                    po = [pall.tile([65, 512], dt.float32, tag=f"po{h}",
                                    name=f"{R}po{hp}_{nch}_{h}") for h in range(2)]
                    pts = {}
                    for mt in range(33):
                        if mt < 32:
                            msl = slice(mt * 128, (mt + 1) * 128)
                            ps_s = pall.tile([128, 2, 512], dt.float32, bufs=2,
                                             tag="ps_s", name=f"{R}ps_s{hp}_{nch}_{mt}")
                            nc.tensor.matmul(ps_s[:, 0, :], kT[0:64, msl], qT[0:64, nsl],
                                             start=True, stop=True)
                            nc.tensor.matmul(ps_s[:, 1, :], kT[64:128, msl], qT[64:128, nsl],
                                             start=True, stop=True, tile_position=(64, 0))
                            pt = ptp.tile([128, 2, 512], dt.bfloat16, bufs=4, tag="pt",
                                          name=f"{R}pt{hp}_{nch}_{mt}")
                            nc.scalar.activation(pt, ps_s, AF.Exp, scale=float(D) ** -0.5)
                            pts[mt] = pt
                        if mt >= 1:
                            ptm = pts.pop(mt - 1)
                            for h in range(2):
                                nc.tensor.matmul(po[h], v_sb[:, mt - 1, 2 * hp + h, :],
                                                 ptm[:, h, :], start=(mt - 1 == 0), stop=(mt - 1 == 31))
                    for h in range(2):
                        ao_dst = ao_lo if h == 0 else ao_hi
                        rden = statp.tile([128, 512], dt.float32, bufs=2,
                                          tag="rden", name=f"{R}rden{hp}_{nch}_{h}")
                        nc.vector.reciprocal(rden[64:65, :], po[h][64:65, :])
                        bc_ps = pall.tile([64, 512], dt.float32, bufs=2, tag="psmall",
                                          name=f"{R}bc{hp}_{nch}_{h}")
                        nc.tensor.matmul(bc_ps, ones_sb[64:65, :], rden[64:65, :],
                                         start=True, stop=True, tile_position=(64, 0))
                        bc_sb = statp.tile([64, 512], dt.float32, bufs=2,
                                           tag="bcs", name=f"{R}bcs{hp}_{nch}_{h}")
                        nc.vector.tensor_copy(bc_sb, bc_ps)
                        nc.vector.tensor_tensor(out=ao_dst[:, hp, nsl],
                                                in0=po[h][0:64, :], in1=bc_sb, op=ALU.mult)

            # ======== phase A: LN1 + QKV (xT scoped) ========
            kqt = {}
            with tc.tile_pool(name=R + "pA", bufs=1) as pA:
                wq_sb = pA.tile([128, 4, C], dt.bfloat16, name=R + "wq_sb")
                wk_sb = pA.tile([128, 4, C], dt.bfloat16, name=R + "wk_sb")
                wv_sb = pA.tile([128, 4, C], dt.bfloat16, name=R + "wv_sb")
                nc.gpsimd.dma_start(out=wq_sb, in_=wq_d[:])
                nc.gpsimd.dma_start(out=wk_sb, in_=wk_d[:])
                nc.gpsimd.dma_start(out=wv_sb, in_=wv_d[:])
                xT = pA.tile([128, 32, 4, 128], dt.bfloat16, name=R + "xT")
                xqT = pA.tile([128, 8, 4, 128], dt.bfloat16, name=R + "xqT")
                kT0 = kqp.tile([128, N], dt.bfloat16, tag="kT", name=f"{R}kT0")
                qT0 = kqp.tile([128, NQ], dt.bfloat16, tag="qT", name=f"{R}qT0")
                kqt[0] = (kT0, qT0)

                def emit_v(mt):
                    ps_v = pall.tile([128, C], dt.float32, bufs=2, tag="psmall",
                                     name=f"{R}ps_v{mt}")
                    for kt in range(4):
                        nc.tensor.matmul(ps_v, xT[:, mt, kt, :],
                                         wv_sb[:, kt, :], start=(kt == 0), stop=(kt == 3))
                    nc.vector.tensor_tensor(
                        out=v_sb[:, mt, :, 0:64],
                        in0=ps_v.rearrange("p (h d) -> p h d", h=H),
                        in1=bv_sb, op=ALU.add)

                def emit_k_chunk(hp, kT, nch):
                    ps_k = pall.tile([128, 512], dt.float32, bufs=2, tag="psmall",
                                     name=f"{R}ps_k{hp}_{nch}")
                    for kt in range(4):
                        nc.tensor.matmul(ps_k, wk_sb[:, kt, hp * 128:(hp + 1) * 128],
                                         xT[:, 4 * nch:4 * nch + 4, kt, :],
                                         start=(kt == 0), stop=(kt == 3))
                    nc.vector.tensor_scalar(kT[:, nch * 512:(nch + 1) * 512],
                                            ps_k, bk_sb[:, hp:hp + 1], None, ALU.add)

                xq4_t = xq.rearrange("(gr j p) c -> p gr j c", p=128, j=4)
                for g in range(2):
                    ln_group(xq4_t[:, g, :, :], g, xqT, "lq")
                for nch in range(2):
                    ps_q = pall.tile([128, 512], dt.float32, bufs=2, tag="psmall",
                                     name=f"{R}ps_q0_{nch}")
                    for kt in range(4):
                        nc.tensor.matmul(ps_q, wq_sb[:, kt, 0:128],
                                         xqT[:, 4 * nch:4 * nch + 4, kt, :],
                                         start=(kt == 0), stop=(kt == 3))
                    nc.vector.tensor_scalar(qT0[:, nch * 512:(nch + 1) * 512],
                                            ps_q, bq_sb[:, 0:1], None, ALU.add)
                xf4_t = xfull.rearrange("(gr j p) c -> p gr j c", p=128, j=4)
                for g in range(8):
                    ln_group(xf4_t[:, g, :, :], g, xT, "l1")
                    emit_k_chunk(0, kT0, g)
                    for mt in range(4 * g, 4 * g + 4):
                        emit_v(mt)
                for hp in range(1, 4):
                    kT = kqp.tile([128, N], dt.bfloat16, tag="kT", name=f"{R}kT{hp}")
                    qT = kqp.tile([128, NQ], dt.bfloat16, tag="qT", name=f"{R}qT{hp}")
                    kqt[hp] = (kT, qT)
                    emit_kq(hp, kT, qT, wk_sb, wq_sb, xT, xqT)
                    emit_attention(hp - 1, *kqt[hp - 1])

            # ======== phase B tail: last attention + wp ========
            emit_attention(3, *kqt[3])
            with tc.tile_pool(name=R + "pC", bufs=1) as pC:
                w1_sb = pC.tile([128, 4, HID], dt.bfloat16, name=R + "w1_sb")
                w2_sb = pC.tile([128, 16, C], dt.bfloat16, name=R + "w2_sb")
                nc.gpsimd.dma_start(out=w1_sb, in_=w1_d[:])
                nc.gpsimd.dma_start(out=w2_sb, in_=w2_d[:])
                for ns in range(8):
                    ps_p = pall.tile([128, C], dt.float32, bufs=2, tag="psmall",
                                     name=f"{R}ps_p{ns}")
                    qsl = slice(ns * 128, (ns + 1) * 128)
                    for hp in range(4):
                        nc.tensor.matmul(ps_p, ao_lo[:, hp, qsl], wp_sb[:, 2 * hp, :],
                                         start=(hp == 0), stop=False)
                        nc.tensor.matmul(ps_p, ao_hi[:, hp, qsl], wp_sb[:, 2 * hp + 1, :],
                                         start=False, stop=(hp == 3))
                    nc.vector.tensor_tensor(out=xq_sb[:, ns, :], in0=xq_sb[:, ns, :],
                                            in1=ps_p, op=ALU.add)
                    nc.vector.tensor_tensor(out=xq_sb[:, ns, :], in0=xq_sb[:, ns, :],
                                            in1=bp_sb, op=ALU.add)

                # ======== phase C: LN2 + MLP ========
                x2T = pC.tile([128, 8, 4, 128], dt.bfloat16, name=R + "x2T")
                for g in range(2):
                    ln_group(xq_sb.rearrange("p (gr j) c -> p gr j c", j=4)[:, g, :, :],
                             g, x2T, "l2", from_sbuf=True)
                for nch in range(2):
                    h_sb = pC.tile([128, 16, 512], dt.bfloat16, tag="h_sb",
                                   name=f"{R}h_sb{nch}")
                    for pt_i in range(16):
                        ps_h = pall.tile([128, 512], dt.float32, bufs=2, tag="psmall",
                                         name=f"{R}ps_h{pt_i}_{nch}")
                        for kt in range(4):
                            nc.tensor.matmul(ps_h, w1_sb[:, kt, pt_i * 128:(pt_i + 1) * 128],
                                             x2T[:, 4 * nch:4 * nch + 4, kt, :],
                                             start=(kt == 0), stop=(kt == 3))
                        nc.vector.tensor_scalar(h_sb[:, pt_i, :],
                                                ps_h, b1_sb[:, pt_i:pt_i + 1], 0.0,
                                                ALU.add, ALU.max)
                    for ns in range(4 * nch, 4 * nch + 4):
                        ps_m = pall.tile([128, C], dt.float32, bufs=2, tag="psmall",
                                         name=f"{R}ps_m{ns}")
                        qsl = slice((ns - 4 * nch) * 128, (ns - 4 * nch + 1) * 128)
                        for kt in range(16):
                            nc.tensor.matmul(ps_m, h_sb[:, kt, qsl], w2_sb[:, kt, :],
                                             start=(kt == 0), stop=(kt == 15))
                        ot = stream.tile([128, C], dt.float32, tag="out", name=f"{R}out{ns}")
                        nc.vector.tensor_tensor(out=ot, in0=ps_m, in1=xq_sb[:, ns, :], op=ALU.add)
                        nc.vector.tensor_tensor(out=ot, in0=ot, in1=b2_sb, op=ALU.add)
                        nc.sync.dma_start(out=y_t[:, ns, :], in_=ot)

    nc.finalize()
    return nc


def _prepare_host(inputs):
    f32 = np.float32
    x = np.asarray(inputs["x"], f32)
    ln1_w = np.asarray(inputs["ln1_w"], f32); ln1_b = np.asarray(inputs["ln1_b"], f32)
    ln2_w = np.asarray(inputs["ln2_w"], f32); ln2_b = np.asarray(inputs["ln2_b"], f32)
    wq = np.asarray(inputs["wq"], f32); wkv = np.asarray(inputs["wkv"], f32)
    wp = np.asarray(inputs["wp"], f32); bp = np.asarray(inputs["bp"], f32)
    w1 = np.asarray(inputs["w1"], f32); b1 = np.asarray(inputs["b1"], f32)
    w2 = np.asarray(inputs["w2"], f32); b2 = np.asarray(inputs["b2"], f32)

    wq_f = ln1_w[:, None] * wq
    wkv_f = ln1_w[:, None] * wkv
    w1_f = ln2_w[:, None] * w1
    bq_f = ln1_b @ wq
    bkv_f = ln1_b @ wkv
    b1_f = b1 + ln2_b @ w1

    def kmaj(w, cols, kt):
        return np.ascontiguousarray(w.reshape(kt, 128, cols).transpose(1, 0, 2)).astype(BF)

    shared = dict(
        wq_d=kmaj(wq_f, C, 4),
        wk_d=kmaj(wkv_f[:, :C], C, 4),
        wv_d=kmaj(wkv_f[:, C:], C, 4),
        wp_d=np.ascontiguousarray(wp.reshape(H, D, C).transpose(1, 0, 2)).astype(BF),
        w1_d=kmaj(w1_f, HID, 4),
        w2_d=np.ascontiguousarray(w2.reshape(16, 128, C).transpose(1, 0, 2)).astype(BF),
        bq_d=np.ascontiguousarray(bq_f.reshape(4, 128).T).astype(f32),
        bk_d=np.ascontiguousarray(bkv_f[:C].reshape(4, 128).T).astype(f32),
        bv_d=np.ascontiguousarray(bkv_f[C:]).astype(f32),
        bp_d=np.ascontiguousarray(bp).astype(f32),
        b1_d=np.ascontiguousarray(b1_f.reshape(16, 128).T).astype(f32),
        b2_d=np.ascontiguousarray(b2).astype(f32),
    )

    in_maps = []
    for core in range(NCORES):
        bi, qi = divmod(core, 4)
        in_maps.append(dict(shared,
                            xfull=np.ascontiguousarray(x[bi]),
                            xq=np.ascontiguousarray(x[bi, qi * NQ:(qi + 1) * NQ])))
    return in_maps


def _make_runner(nc):
    """Persistent jitted SPMD executor for `nc` (mirrors bass2jax.run_bass_via_pjrt
    but keeps the jitted function + avoids per-call retrace)."""
    import jax
    import numpy as jnp_np
    from jax.sharding import Mesh, PartitionSpec
    from jax.experimental.shard_map import shard_map
    import concourse.mybir as mybir
    from concourse import bass2jax

    bass2jax.install_neuronx_cc_hook()

    partition_name = nc.partition_id_tensor.name if nc.partition_id_tensor else None
    in_names, out_names, out_avals = [], [], []
    for alloc in nc.m.functions[0].allocations:
        if not isinstance(alloc, mybir.MemoryLocationSet):
            continue
        name = alloc.memorylocations[0].name
        if alloc.kind == "ExternalInput":
            if name != partition_name:
                in_names.append(name)
        elif alloc.kind == "ExternalOutput":
            out_names.append(name)
            out_avals.append(jax.core.ShapedArray(tuple(alloc.tensor_shape),
                                                  mybir.dt.np(alloc.dtype)))
    n_params = len(in_names)
    all_names = in_names + out_names
    if partition_name is not None:
        all_names = all_names + [partition_name]

    def _body(*args):
        operands = list(args)
        if partition_name is not None:
            operands.append(bass2jax.partition_id_tensor())
        outs = bass2jax._bass_exec_p.bind(
            *operands,
            out_avals=tuple(out_avals),
            in_names=tuple(all_names),
            out_names=tuple(out_names),
            lowering_input_output_aliases=(),
            sim_require_finite=True,
            sim_require_nnan=True,
            nc=nc,
        )
        return tuple(outs)

    devices = jax.devices()[:NCORES]
    mesh = Mesh(np.asarray(devices), ("core",))
    n_outs = len(out_names)
    sharded = jax.jit(
        shard_map(_body, mesh=mesh,
                  in_specs=(PartitionSpec("core"),) * (n_params + n_outs),
                  out_specs=(PartitionSpec("core"),) * n_outs,
                  check_rep=False),
        keep_unused=True,
    )

    def run(in_maps):
        concat_in = [np.concatenate([np.asarray(in_maps[c][name]) for c in range(NCORES)], axis=0)
                     for name in in_names]
        zeros = [np.zeros((NCORES * a.shape[0], *a.shape[1:]), a.dtype) for a in out_avals]
        out_arrs = sharded(*concat_in, *zeros)
        return [{name: np.asarray(out_arrs[i]).reshape(NCORES, *out_avals[i].shape)[c]
                 for i, name in enumerate(out_names)}
                for c in range(NCORES)]

    run.sharded = sharded
    run.in_names = in_names
    run.out_names = out_names
    run.out_avals = out_avals
    return run


def get_runner(repeat=1):
    key = f"runner{repeat}"
    if key not in _CACHE:
        _CACHE[key] = _make_runner(_build_program(repeat=repeat))
    return _CACHE[key]


def kernel(**inputs):
    runner = get_runner()
    in_maps = _prepare_host(inputs)
    results = runner(in_maps)
    out = np.empty((B, N, C), np.float32)
    for core in range(NCORES):
        bi, qi = divmod(core, 4)
        out[bi, qi * NQ:(qi + 1) * NQ] = results[core]["y"]
    return out



# revision 1
# speedup vs baseline: 2.5053x; 2.5053x over previous
"""Trainium2 Bass kernel for a pre-norm transformer block (attention + MLP).

Sharding: batch (2) x query-block (4) across 8 cores. Each core computes
LN1 + K/V over its full batch (replicated within its 4-core group) and
attention / projection / MLP for its own 1024 query tokens. No collectives.

Device layouts (per core):
  xT  : LN1(x) feature-major [128c, 4ct, 4096t] bf16
  kT  : per head-pair hp [128 (2 heads x 64d), 4096m] bf16
  v   : token-major [128m, 32mt, 8h, 65] bf16 (65th col = ones -> softmax denom)
  S^T : [128m, 2h, 512n] PSUM (keys on partitions; head pair row-packed)
  A@V : out^T[65, n]: lhsT=[V|1] per head, accumulated over 32 m-tiles
"""

import numpy as np
import ml_dtypes

B, N, C = 2, 4096, 512
H, D = 8, 64
HID = 2048
NQ = 1024
NCORES = 8
EPS = 1e-5
BF = ml_dtypes.bfloat16

_CACHE = {}


def _build_program(repeat=1):
    from concourse import bacc
    import concourse.bass as bass
    import concourse.mybir as mybir
    from concourse.tile import TileContext

    dt = mybir.dt
    AF = mybir.ActivationFunctionType
    ALU = mybir.AluOpType

    nc = bacc.Bacc(None, target_bir_lowering=False)

    xfull = nc.dram_tensor("xfull", (N, C), dt.float32, kind="ExternalInput")
    xq = nc.dram_tensor("xq", (NQ, C), dt.float32, kind="ExternalInput")
    wq_d = nc.dram_tensor("wq_d", (128, 4, C), dt.bfloat16, kind="ExternalInput")
    wk_d = nc.dram_tensor("wk_d", (128, 4, C), dt.bfloat16, kind="ExternalInput")
    wv_d = nc.dram_tensor("wv_d", (128, 4, C), dt.bfloat16, kind="ExternalInput")
    wp_d = nc.dram_tensor("wp_d", (64, 8, C), dt.bfloat16, kind="ExternalInput")
    w1_d = nc.dram_tensor("w1_d", (128, 4, HID), dt.bfloat16, kind="ExternalInput")
    w2_d = nc.dram_tensor("w2_d", (128, 16, C), dt.bfloat16, kind="ExternalInput")
    bq_d = nc.dram_tensor("bq_d", (128, 4), dt.float32, kind="ExternalInput")
    bk_d = nc.dram_tensor("bk_d", (128, 4), dt.float32, kind="ExternalInput")
    bv_d = nc.dram_tensor("bv_d", (C,), dt.float32, kind="ExternalInput")
    bp_d = nc.dram_tensor("bp_d", (C,), dt.float32, kind="ExternalInput")
    b1_d = nc.dram_tensor("b1_d", (128, 16), dt.float32, kind="ExternalInput")
    b2_d = nc.dram_tensor("b2_d", (C,), dt.float32, kind="ExternalInput")
    y = nc.dram_tensor("y", (NQ, C), dt.float32, kind="ExternalOutput")

    xfull_t = xfull.rearrange("(i p) c -> p i c", p=128)
    xq_t = xq.rearrange("(i p) c -> p i c", p=128)
    y_t = y.rearrange("(i p) c -> p i c", p=128)

    import contextlib
    with TileContext(nc) as tc:
      with (tc.For_i(0, repeat, 1) if repeat > 1 else contextlib.nullcontext()):
        R = "r0_"
        with tc.tile_pool(name=R + "pers", bufs=1) as pers, \
             tc.tile_pool(name=R + "stat", bufs=4) as statp, \
             tc.tile_pool(name=R + "stream", bufs=3) as stream, \
             tc.tile_pool(name=R + "pB", bufs=1) as pB, \
             tc.tile_pool(name=R + "kq", bufs=2) as kqp, \
             tc.tile_pool(name=R + "ptp", bufs=3) as ptp, \
             tc.tile_pool(name=R + "pall", bufs=1, space="PSUM") as pall:

            eps_t = pers.tile([128, 1], dt.float32, name=R + "eps")
            nc.vector.memset(eps_t, EPS)
            ones_sb = pers.tile([128, 64], dt.float32, name=R + "ones")
            nc.vector.memset(ones_sb, 1.0)
            xq_sb = pers.tile([128, 8, C], dt.float32, name=R + "xq_sb")
            nc.sync.dma_start(out=xq_sb, in_=xq_t[:])
            bq_sb = pers.tile([128, 4], dt.float32, name=R + "bq_sb")
            bk_sb = pers.tile([128, 4], dt.float32, name=R + "bk_sb")
            bv_sb = pers.tile([128, 8, 64], dt.bfloat16, name=R + "bv_sb")
            bp_sb = pers.tile([128, C], dt.float32, name=R + "bp_sb")
            b1_sb = pers.tile([128, 16], dt.float32, name=R + "b1_sb")
            b2_sb = pers.tile([128, C], dt.float32, name=R + "b2_sb")
            nc.sync.dma_start(out=bq_sb, in_=bq_d[:])
            nc.sync.dma_start(out=bk_sb, in_=bk_d[:])
            nc.sync.dma_start(out=b1_sb, in_=b1_d[:])
            nc.gpsimd.dma_start(out=bv_sb, in_=bass.AP(tensor=bv_d, offset=0, ap=[[0, 128], [1, C]]))
            nc.gpsimd.dma_start(out=bp_sb, in_=bass.AP(tensor=bp_d, offset=0, ap=[[0, 128], [1, C]]))
            nc.gpsimd.dma_start(out=b2_sb, in_=bass.AP(tensor=b2_d, offset=0, ap=[[0, 128], [1, C]]))

            # phase-B persistent tiles (attention)
            wp_sb = pB.tile([64, 8, C], dt.bfloat16, name=R + "wp_sb")
            nc.gpsimd.dma_start(out=wp_sb, in_=wp_d[:])
            v_sb = pB.tile([128, 32, H, 65], dt.bfloat16, name=R + "v_sb")
            nc.vector.memset(v_sb[:, :, :, 64:65], 1.0)
            ao_lo = pB.tile([64, 4, NQ], dt.bfloat16, name=R + "ao_lo")
            ao_hi = pB.tile([64, 4, NQ], dt.bfloat16, name=R + "ao_hi")

            def ln_group(src_dram_or_sb, g, xT, tag, from_sbuf=False):
                """4 token-tiles: 1 load, per-tile LN stats+normalize, 1 batched transpose.

                xT layout: [128 ci, nblk, 4 co, 128 t]."""
                eng = nc.sync if g % 2 == 0 else nc.scalar
                if from_sbuf:
                    xt4 = src_dram_or_sb
                else:
                    xt4 = stream.tile([128, 4, C], dt.float32, tag="lnx", bufs=2, name=f"{R}{tag}x{g}")
                    eng.dma_start(out=xt4, in_=src_dram_or_sb)
                xn4 = stream.tile([128, 4, C], dt.bfloat16, tag="lnn", bufs=3, name=f"{R}{tag}n{g}")
                for j in range(4):
                    i = 4 * g + j
                    stats = statp.tile([128, 6], dt.float32, tag="lnst", name=f"{R}{tag}st{i}")
                    mv = statp.tile([128, 2], dt.float32, tag="lnmv", name=f"{R}{tag}mv{i}")
                    nc.vector.bn_stats(stats, xt4[:, j, :])
                    nc.vector.bn_aggr(mv, stats)
                    sq = statp.tile([128, 1], dt.float32, tag="lnsq", name=f"{R}{tag}sq{i}")
                    nc.scalar.activation(sq, mv[:, 1:2], AF.Sqrt, bias=eps_t)
                    rstd = statp.tile([128, 1], dt.float32, tag="lnrs", name=f"{R}{tag}rs{i}")
                    nc.vector.reciprocal(rstd, sq)
                    nmr = statp.tile([128, 1], dt.float32, tag="lnnm", name=f"{R}{tag}nm{i}")
                    nc.vector.tensor_scalar(nmr, mv[:, 0:1], rstd, -1.0, ALU.mult, ALU.mult)
                    nc.scalar.activation(xn4[:, j, :], xt4[:, j, :], AF.Identity, bias=nmr, scale=rstd)
                eng.dma_start(out=xT[:, 4 * g:4 * g + 4, :, :], in_=xn4, transpose=True)

            def emit_kq(hp, kT, qT, wk_sb, wq_sb, xT, xqT):
                for nch in range(8):
                    ps_k = pall.tile([128, 512], dt.float32, bufs=2, tag="psmall",
                                     name=f"{R}ps_k{hp}_{nch}")
                    for kt in range(4):
                        nc.tensor.matmul(ps_k, wk_sb[:, kt, hp * 128:(hp + 1) * 128],
                                         xT[:, 4 * nch:4 * nch + 4, kt, :],
                                         start=(kt == 0), stop=(kt == 3))
                    nc.vector.tensor_scalar(kT[:, nch * 512:(nch + 1) * 512],
                                            ps_k, bk_sb[:, hp:hp + 1], None, ALU.add)
                for nch in range(2):
                    ps_q = pall.tile([128, 512], dt.float32, bufs=2, tag="psmall",
                                     name=f"{R}ps_q{hp}_{nch}")
                    for kt in range(4):
                        nc.tensor.matmul(ps_q, wq_sb[:, kt, hp * 128:(hp + 1) * 128],
                                         xqT[:, 4 * nch:4 * nch + 4, kt, :],
                                         start=(kt == 0), stop=(kt == 3))
                    nc.vector.tensor_scalar(qT[:, nch * 512:(nch + 1) * 512],
                                            ps_q, bq_sb[:, hp:hp + 1], None, ALU.add)

            def emit_attention(hp, kT, qT):
                for nch in range(2):
                    nsl = slice(nch * 512, (nch + 1) * 512)
                    po = [pall.tile([65, 512], dt.float32, tag=f"po{h}",
                                    name=f"{R}po{hp}_{nch}_{h}") for h in range(2)]
                    pts = {}
                    for mt in range(33):
                        if mt < 32:
                            msl = slice(mt * 128, (mt + 1) * 128)
                            ps_s = pall.tile([128, 2, 512], dt.float32, bufs=2,
                                             tag="ps_s", name=f"{R}ps_s{hp}_{nch}_{mt}")
                            nc.tensor.matmul(ps_s[:, 0, :], kT[0:64, msl], qT[0:64, nsl],
                                             start=True, stop=True)
                            nc.tensor.matmul(ps_s[:, 1, :], kT[64:128, msl], qT[64:128, nsl],
                                             start=True, stop=True, tile_position=(64, 0))
                            pt = ptp.tile([128, 2, 512], dt.bfloat16, bufs=4, tag="pt",
                                          name=f"{R}pt{hp}_{nch}_{mt}")
                            nc.scalar.activation(pt, ps_s, AF.Exp, scale=float(D) ** -0.5)
                            pts[mt] = pt
                        if mt >= 1:
                            ptm = pts.pop(mt - 1)
                            for h in range(2):
                                nc.tensor.matmul(po[h], v_sb[:, mt - 1, 2 * hp + h, :],
                                                 ptm[:, h, :], start=(mt - 1 == 0), stop=(mt - 1 == 31))
                    for h in range(2):
                        ao_dst = ao_lo if h == 0 else ao_hi
                        rden = statp.tile([128, 512], dt.float32, bufs=2,
                                          tag="rden", name=f"{R}rden{hp}_{nch}_{h}")
                        nc.vector.reciprocal(rden[64:65, :], po[h][64:65, :])
                        bc_ps = pall.tile([64, 512], dt.float32, bufs=2, tag="psmall",
                                          name=f"{R}bc{hp}_{nch}_{h}")
                        nc.tensor.matmul(bc_ps, ones_sb[64:65, :], rden[64:65, :],
                                         start=True, stop=True, tile_position=(64, 0))
                        bc_sb = statp.tile([64, 512], dt.float32, bufs=2,
                                           tag="bcs", name=f"{R}bcs{hp}_{nch}_{h}")
                        nc.vector.tensor_copy(bc_sb, bc_ps)
                        nc.vector.tensor_tensor(out=ao_dst[:, hp, nsl],
                                                in0=po[h][0:64, :], in1=bc_sb, op=ALU.mult)

            # ======== phase A: LN1 + QKV (xT scoped) ========
            kqt = {}
            with tc.tile_pool(name=R + "pA", bufs=1) as pA:
                wq_sb = pA.tile([128, 4, C], dt.bfloat16, name=R + "wq_sb")
                wk_sb = pA.tile([128, 4, C], dt.bfloat16, name=R + "wk_sb")
                wv_sb = pA.tile([128, 4, C], dt.bfloat16, name=R + "wv_sb")
                nc.gpsimd.dma_start(out=wq_sb, in_=wq_d[:])
                nc.gpsimd.dma_start(out=wk_sb, in_=wk_d[:])
                nc.gpsimd.dma_start(out=wv_sb, in_=wv_d[:])
                xT = pA.tile([128, 32, 4, 128], dt.bfloat16, name=R + "xT")
                xqT = pA.tile([128, 8, 4, 128], dt.bfloat16, name=R + "xqT")
                kT0 = kqp.tile([128, N], dt.bfloat16, tag="kT", name=f"{R}kT0")
                qT0 = kqp.tile([128, NQ], dt.bfloat16, tag="qT", name=f"{R}qT0")
                kqt[0] = (kT0, qT0)

                def emit_v(mt):
                    ps_v = pall.tile([128, C], dt.float32, bufs=2, tag="psmall",
                                     name=f"{R}ps_v{mt}")
                    for kt in range(4):
                        nc.tensor.matmul(ps_v, xT[:, mt, kt, :],
                                         wv_sb[:, kt, :], start=(kt == 0), stop=(kt == 3))
                    nc.vector.tensor_tensor(
                        out=v_sb[:, mt, :, 0:64],
                        in0=ps_v.rearrange("p (h d) -> p h d", h=H),
                        in1=bv_sb, op=ALU.add)

                def emit_k_chunk(hp, kT, nch):
                    ps_k = pall.tile([128, 512], dt.float32, bufs=2, tag="psmall",
                                     name=f"{R}ps_k{hp}_{nch}")
                    for kt in range(4):
                        nc.tensor.matmul(ps_k, wk_sb[:, kt, hp * 128:(hp + 1) * 128],
                                         xT[:, 4 * nch:4 * nch + 4, kt, :],
                                         start=(kt == 0), stop=(kt == 3))
                    nc.vector.tensor_scalar(kT[:, nch * 512:(nch + 1) * 512],
                                            ps_k, bk_sb[:, hp:hp + 1], None, ALU.add)

                xq4_t = xq.rearrange("(gr j p) c -> p gr j c", p=128, j=4)
                for g in range(2):
                    ln_group(xq4_t[:, g, :, :], g, xqT, "lq")
                for nch in range(2):
                    ps_q = pall.tile([128, 512], dt.float32, bufs=2, tag="psmall",
                                     name=f"{R}ps_q0_{nch}")
                    for kt in range(4):
                        nc.tensor.matmul(ps_q, wq_sb[:, kt, 0:128],
                                         xqT[:, 4 * nch:4 * nch + 4, kt, :],
                                         start=(kt == 0), stop=(kt == 3))
                    nc.vector.tensor_scalar(qT0[:, nch * 512:(nch + 1) * 512],
                                            ps_q, bq_sb[:, 0:1], None, ALU.add)
                xf4_t = xfull.rearrange("(gr j p) c -> p gr j c", p=128, j=4)
                for g in range(8):
                    ln_group(xf4_t[:, g, :, :], g, xT, "l1")
                    emit_k_chunk(0, kT0, g)
                    for mt in range(4 * g, 4 * g + 4):
                        emit_v(mt)
                for hp in range(1, 4):
                    kT = kqp.tile([128, N], dt.bfloat16, tag="kT", name=f"{R}kT{hp}")
                    qT = kqp.tile([128, NQ], dt.bfloat16, tag="qT", name=f"{R}qT{hp}")
                    kqt[hp] = (kT, qT)
                    emit_kq(hp, kT, qT, wk_sb, wq_sb, xT, xqT)
                    emit_attention(hp - 1, *kqt[hp - 1])

            # ======== phase B tail: last attention + wp ========
            emit_attention(3, *kqt[3])
            with tc.tile_pool(name=R + "pC", bufs=1) as pC:
                w1_sb = pC.tile([128, 4, HID], dt.bfloat16, name=R + "w1_sb")
                w2_sb = pC.tile([128, 16, C], dt.bfloat16, name=R + "w2_sb")
                nc.gpsimd.dma_start(out=w1_sb, in_=w1_d[:])
                nc.gpsimd.dma_start(out=w2_sb, in_=w2_d[:])
                for ns in range(8):
                    ps_p = pall.tile([128, C], dt.float32, bufs=2, tag="psmall",
                                     name=f"{R}ps_p{ns}")
                    qsl = slice(ns * 128, (ns + 1) * 128)
                    for hp in range(4):
                        nc.tensor.matmul(ps_p, ao_lo[:, hp, qsl], wp_sb[:, 2 * hp, :],
                                         start=(hp == 0), stop=False)
                        nc.tensor.matmul(ps_p, ao_hi[:, hp, qsl], wp_sb[:, 2 * hp + 1, :],
                                         start=False, stop=(hp == 3))
                    nc.vector.tensor_tensor(out=xq_sb[:, ns, :], in0=xq_sb[:, ns, :],
                                            in1=ps_p, op=ALU.add)
                    nc.vector.tensor_tensor(out=xq_sb[:, ns, :], in0=xq_sb[:, ns, :],
                                            in1=bp_sb, op=ALU.add)

                # ======== phase C: LN2 + MLP ========
                x2T = pC.tile([128, 8, 4, 128], dt.bfloat16, name=R + "x2T")
                for g in range(2):
                    ln_group(xq_sb.rearrange("p (gr j) c -> p gr j c", j=4)[:, g, :, :],
                             g, x2T, "l2", from_sbuf=True)
                for nch in range(2):
                    h_sb = pC.tile([128, 16, 512], dt.bfloat16, tag="h_sb",
                                   name=f"{R}h_sb{nch}")
                    for pt_i in range(16):
                        ps_h = pall.tile([128, 512], dt.float32, bufs=2, tag="psmall",
                                         name=f"{R}ps_h{pt_i}_{nch}")
                        for kt in range(4):
                            nc.tensor.matmul(ps_h, w1_sb[:, kt, pt_i * 128:(pt_i + 1) * 128],
                                             x2T[:, 4 * nch:4 * nch + 4, kt, :],
                                             start=(kt == 0), stop=(kt == 3))
                        nc.vector.tensor_scalar(h_sb[:, pt_i, :],
                                                ps_h, b1_sb[:, pt_i:pt_i + 1], 0.0,
                                                ALU.add, ALU.max)
                    for ns in range(4 * nch, 4 * nch + 4):
                        ps_m = pall.tile([128, C], dt.float32, bufs=2, tag="psmall",
                                         name=f"{R}ps_m{ns}")
                        qsl = slice((ns - 4 * nch) * 128, (ns - 4 * nch + 1) * 128)
                        for kt in range(16):
                            nc.tensor.matmul(ps_m, h_sb[:, kt, qsl], w2_sb[:, kt, :],
                                             start=(kt == 0), stop=(kt == 15))
                        ot = stream.tile([128, C], dt.float32, tag="out", name=f"{R}out{ns}")
                        nc.vector.tensor_tensor(out=ot, in0=ps_m, in1=xq_sb[:, ns, :], op=ALU.add)
                        nc.vector.tensor_tensor(out=ot, in0=ot, in1=b2_sb, op=ALU.add)
                        nc.sync.dma_start(out=y_t[:, ns, :], in_=ot)

    nc.finalize()
    return nc


def _prepare_host(inputs):
    f32 = np.float32
    x = np.asarray(inputs["x"], f32)
    ln1_w = np.asarray(inputs["ln1_w"], f32); ln1_b = np.asarray(inputs["ln1_b"], f32)
    ln2_w = np.asarray(inputs["ln2_w"], f32); ln2_b = np.asarray(inputs["ln2_b"], f32)
    wq = np.asarray(inputs["wq"], f32); wkv = np.asarray(inputs["wkv"], f32)
    wp = np.asarray(inputs["wp"], f32); bp = np.asarray(inputs["bp"], f32)
    w1 = np.asarray(inputs["w1"], f32); b1 = np.asarray(inputs["b1"], f32)
    w2 = np.asarray(inputs["w2"], f32); b2 = np.asarray(inputs["b2"], f32)

    wq_f = ln1_w[:, None] * wq
    wkv_f = ln1_w[:, None] * wkv
    w1_f = ln2_w[:, None] * w1
    bq_f = ln1_b @ wq
    bkv_f = ln1_b @ wkv
    b1_f = b1 + ln2_b @ w1

    def kmaj(w, cols, kt):
        return np.ascontiguousarray(w.reshape(kt, 128, cols).transpose(1, 0, 2)).astype(BF)

    shared = dict(
        wq_d=kmaj(wq_f, C, 4),
        wk_d=kmaj(wkv_f[:, :C], C, 4),
        wv_d=kmaj(wkv_f[:, C:], C, 4),
        wp_d=np.ascontiguousarray(wp.reshape(H, D, C).transpose(1, 0, 2)).astype(BF),
        w1_d=kmaj(w1_f, HID, 4),
        w2_d=np.ascontiguousarray(w2.reshape(16, 128, C).transpose(1, 0, 2)).astype(BF),
        bq_d=np.ascontiguousarray(bq_f.reshape(4, 128).T).astype(f32),
        bk_d=np.ascontiguousarray(bkv_f[:C].reshape(4, 128).T).astype(f32),
        bv_d=np.ascontiguousarray(bkv_f[C:]).astype(f32),
        bp_d=np.ascontiguousarray(bp).astype(f32),
        b1_d=np.ascontiguousarray(b1_f.reshape(16, 128).T).astype(f32),
        b2_d=np.ascontiguousarray(b2).astype(f32),
    )

    in_maps = []
    for core in range(NCORES):
        bi, qi = divmod(core, 4)
        in_maps.append(dict(shared,
                            xfull=np.ascontiguousarray(x[bi]),
                            xq=np.ascontiguousarray(x[bi, qi * NQ:(qi + 1) * NQ])))
    return in_maps


def _make_runner(nc):
    """Persistent jitted SPMD executor for `nc` (mirrors bass2jax.run_bass_via_pjrt
    but keeps the jitted function + avoids per-call retrace)."""
    import jax
    import numpy as jnp_np
    from jax.sharding import Mesh, PartitionSpec
    from jax.experimental.shard_map import shard_map
    import concourse.mybir as mybir
    from concourse import bass2jax

    bass2jax.install_neuronx_cc_hook()

    partition_name = nc.partition_id_tensor.name if nc.partition_id_tensor else None
    in_names, out_names, out_avals = [], [], []
    for alloc in nc.m.functions[0].allocations:
        if not isinstance(alloc, mybir.MemoryLocationSet):
            continue
        name = alloc.memorylocations[0].name
        if alloc.kind == "ExternalInput":
            if name != partition_name:
                in_names.append(name)
        elif alloc.kind == "ExternalOutput":
            out_names.append(name)
            out_avals.append(jax.core.ShapedArray(tuple(alloc.tensor_shape),
                                                  mybir.dt.np(alloc.dtype)))
    n_params = len(in_names)
    all_names = in_names + out_names
    if partition_name is not None:
        all_names = all_names + [partition_name]

    def _body(*args):
        operands = list(args)
        if partition_name is not None:
            operands.append(bass2jax.partition_id_tensor())
        outs = bass2jax._bass_exec_p.bind(
            *operands,
            out_avals=tuple(out_avals),
            in_names=tuple(all_names),
            out_names=tuple(out_names),
            lowering_input_output_aliases=(),
            sim_require_finite=True,
            sim_require_nnan=True,
            nc=nc,
        )
        return tuple(outs)

    devices = jax.devices()[:NCORES]
    mesh = Mesh(np.asarray(devices), ("core",))
    n_outs = len(out_names)
    sharded = jax.jit(
        shard_map(_body, mesh=mesh,
                  in_specs=(PartitionSpec("core"),) * (n_params + n_outs),
                  out_specs=(PartitionSpec("core"),) * n_outs,
                  check_rep=False),
        keep_unused=True,
    )

    def run(in_maps):
        concat_in = [np.concatenate([np.asarray(in_maps[c][name]) for c in range(NCORES)], axis=0)
                     for name in in_names]
        zeros = [np.zeros((NCORES * a.shape[0], *a.shape[1:]), a.dtype) for a in out_avals]
        out_arrs = sharded(*concat_in, *zeros)
        return [{name: np.asarray(out_arrs[i]).reshape(NCORES, *out_avals[i].shape)[c]
                 for i, name in enumerate(out_names)}
                for c in range(NCORES)]

    run.sharded = sharded
    run.in_names = in_names
    run.out_names = out_names
    run.out_avals = out_avals
    return run


def get_runner(repeat=1):
    key = f"runner{repeat}"
    if key not in _CACHE:
        _CACHE[key] = _make_runner(_build_program(repeat=repeat))
    return _CACHE[key]


def kernel(**inputs):
    runner = get_runner()
    in_maps = _prepare_host(inputs)
    results = runner(in_maps)
    out = np.empty((B, N, C), np.float32)
    for core in range(NCORES):
        bi, qi = divmod(core, 4)
        out[bi, qi * NQ:(qi + 1) * NQ] = results[core]["y"]
    return out

